# revision 2
# baseline (speedup 1.0000x reference)
"""Trainium2 Bass kernel for a dense transformer block (single-head attn + MLP).

Sharding: 8 cores; core c handles batch b=c//2, query-half h=c%2.
Each core computes K,V for all 2048 tokens of its batch (no collectives).
Host permutes tokens so each core's query tokens are always columns 0..1023
of its transposed input (SPMD uniform program).

Layout: activations kept transposed [C, T] (channels on partitions) so every
matmul feeds the PE directly.  LN stats via ones-matmuls on PE; per-token row
broadcasts via K=1 outer-product matmuls; softmax without max subtraction
(logits are ~N(0, 0.4^2): verified small); softmax denominator folded into
the y-eviction.  All matmuls in float32r (FP22, full PE rate at N>=256).
Attention scores are computed inside phase A while k^T is still in SBUF;
exp(att) and V spill to DRAM and stream back for the y/proj phase.  The MLP
streams each weight exactly once (H-halves, DMA-accumulated output).
"""

import numpy as np
import concourse.bass as bass
import concourse.mybir as mybir
import concourse.tile as tile
from concourse import bacc
from concourse.bass_utils import run_bass_kernel_spmd

F32 = mybir.dt.float32
F32R = mybir.dt.float32r
AF = mybir.ActivationFunctionType
ALU = mybir.AluOpType

P = 128
C = 1024        # n_embd
T = 2048        # key tokens per batch
TQ = 1024       # query tokens per core
H = 4096        # mlp hidden
CK = C // P     # 8
HK = H // P     # 32
S = T // P      # 16 key tiles
NCH = 512       # matmul moving-dim chunk
EPS = 1e-5
ATT_SCALE = 1.0 / 32.0   # 1/sqrt(C)

N_CORES = 8


def _build(reps=1, debug_taps=False):
    nc = bacc.Bacc()

    xT = nc.declare_dram_parameter("xT", [C, TQ], F32R, isOutput=False)
    w1qk = nc.declare_dram_parameter("w1qk", [2 * CK, P, C], F32R, isOutput=False)
    w1v = nc.declare_dram_parameter("w1v", [CK, P, C], F32R, isOutput=False)
    wp = nc.declare_dram_parameter("wp", [CK, P, C], F32R, isOutput=False)
    w2 = nc.declare_dram_parameter("w2", [HK, P, C], F32R, isOutput=False)
    wm = nc.declare_dram_parameter("wm", [CK, P, H], F32R, isOutput=False)
    c1q = nc.declare_dram_parameter("c1q", [CK, P], F32, isOutput=False)
    c1k = nc.declare_dram_parameter("c1k", [CK, P], F32, isOutput=False)
    c1vb = nc.declare_dram_parameter("c1vb", [P, C], F32, isOutput=False)
    bp = nc.declare_dram_parameter("bp", [CK, P], F32, isOutput=False)
    c2 = nc.declare_dram_parameter("c2", [HK, P], F32, isOutput=False)
    bm = nc.declare_dram_parameter("bm", [CK, P], F32, isOutput=False)
    onc = nc.declare_dram_parameter("onc", [P, 1], F32R, isOutput=False)
    onr = nc.declare_dram_parameter("onr", [1, P], F32R, isOutput=False)
    out_t = nc.declare_dram_parameter("out_t", [C, TQ], F32, isOutput=True)

    kv_self = nc.dram_tensor("kv_self", [2, TQ // P, P, TQ], F32R)
    kv_all = nc.dram_tensor("kv_all", [2, 2, TQ // P, P, TQ], F32R)
    if debug_taps:
        yT_d = nc.dram_tensor("yT_d", [P, CK, TQ], F32)
        x2_d = nc.dram_tensor("x2_d", [P, CK, TQ], F32)
        xh2_d = nc.dram_tensor("xh2_d", [P, CK, TQ], F32)
        gel_d = nc.dram_tensor("gel_d", [P, HK // 2, TQ], F32)
    attd = nc.dram_tensor("attd", [S, P, TQ], F32R)

    xT3 = xT.rearrange("(k p) t -> p k t", p=P)

    with tile.TileContext(nc) as tc:
        with (
            tc.tile_pool(name="glob", bufs=1) as gp,
            tc.tile_pool(name="ps", bufs=8, space="PSUM") as pp,
        ):
            def pst(pdim=P):
                return pp.tile([pdim, NCH], F32, tag="ps", name="ps")

            ones_col = gp.tile([P, 1], F32R)
            nc.sync.dma_start(ones_col[:], onc[:])
            ones_row = gp.tile([1, P], F32R)
            nc.sync.dma_start(ones_row[:], onr[:])
            c1q_t = gp.tile([P, CK], F32)
            nc.sync.dma_start(c1q_t[:], c1q.rearrange("j p -> p j"))
            c1k_t = gp.tile([P, CK], F32)
            nc.sync.dma_start(c1k_t[:], c1k.rearrange("j p -> p j"))
            bp_t = gp.tile([P, CK], F32)
            nc.sync.dma_start(bp_t[:], bp.rearrange("j p -> p j"))
            bm_t = gp.tile([P, CK], F32)
            nc.sync.dma_start(bm_t[:], bm.rearrange("j p -> p j"))
            c2_t = gp.tile([P, HK], F32)
            nc.sync.dma_start(c2_t[:], c2.rearrange("j p -> p j"))
            c1v_t = gp.tile([P, C], F32)
            nc.sync.dma_start(c1v_t[:], c1vb[:])
            recip_b = gp.tile([P, TQ], F32)
            eps_col = gp.tile([P, 1], F32)
            nc.vector.memset(eps_col[:], EPS)

            def ln_center(src3, width, r_b, mu_b, rbase, xc, r_col=None):
                """LN stats over channels of a transposed activation.
                Centers src3 into xc (xc may be src3: in-place); fills
                broadcast rows r_b/mu_b[:, rbase:rbase+width] and the f32r
                reciprocal-sigma row r_row_r[:, rbase:rbase+width].
                Callers fold the * r scaling into their PSUM evictions."""
                for sub in range(width // NCH):
                    lo = sub * NCH
                    mu_ps = pst(1)
                    s2_ps = pst(1)
                    for k in range(CK):
                        nc.tensor.matmul(mu_ps[:], ones_col[:],
                                         src3[:, k, lo:lo + NCH],
                                         start=(k == 0), stop=(k == CK - 1))
                    for k in range(CK):
                        sq = gp.tile([P, NCH], F32R, tag="sq", bufs=2)
                        nc.scalar.activation(
                            sq[:], src3[:, k, lo:lo + NCH].bitcast(F32),
                            AF.Square)
                        nc.tensor.matmul(s2_ps[:], ones_col[:], sq[:],
                                         start=(k == 0), stop=(k == CK - 1))
                    mu_sb = gp.tile([1, NCH], F32R, tag="murow", bufs=2)
                    nc.scalar.activation(mu_sb[:], mu_ps[:], AF.Copy,
                                         scale=1.0 / C)
                    musq = gp.tile([1, NCH], F32, tag="musq", bufs=2)
                    nc.scalar.activation(musq[:], mu_ps[:], AF.Square,
                                         scale=1.0 / C)
                    var_sb = gp.tile([1, NCH], F32, tag="varrow", bufs=2)
                    nc.vector.scalar_tensor_tensor(
                        var_sb[:], s2_ps[:], 1.0 / C, musq[:],
                        op0=ALU.mult, op1=ALU.subtract)
                    # reciprocal-sigma row (f32r for K=1 matmul use)
                    sqv = gp.tile([1, NCH], F32, tag="sqvrow", bufs=2)
                    nc.scalar.activation(sqv[:], var_sb[:], AF.Sqrt,
                                         bias=eps_col[0:1])
                    nc.vector.reciprocal(sqv[:], sqv[:])
                    rrow = gp.tile([1, NCH], F32R, tag="rrow", bufs=2)
                    rr = rrow[:]
                    nc.scalar.activation(rr, sqv[:], AF.Copy)
                    if r_col is not None:
                        for b in range(NCH // P):
                            blk = (rbase + lo) // P + b
                            rc_ps = pst()
                            nc.tensor.matmul(
                                rc_ps[:, 0:P], rr[:, b * P:(b + 1) * P],
                                ones_row[:], start=True, stop=True)
                            nc.vector.tensor_copy(r_col[:, blk:blk + 1],
                                                  rc_ps[:, 0:1])
                    # broadcast r and mu to all partitions
                    rb_ps = pst()
                    nc.tensor.matmul(rb_ps[:], ones_row[:], rr,
                                     start=True, stop=True)
                    nc.vector.tensor_copy(
                        r_b[:, rbase + lo:rbase + lo + NCH], rb_ps[:])
                    mb_ps = pst()
                    nc.tensor.matmul(mb_ps[:], ones_row[:], mu_sb[:],
                                     start=True, stop=True)
                    nc.vector.tensor_copy(
                        mu_b[:, rbase + lo:rbase + lo + NCH], mb_ps[:])
                    for k in range(CK):
                        nc.vector.tensor_sub(
                            xc[:, k, lo:lo + NCH],
                            src3[:, k, lo:lo + NCH].bitcast(F32),
                            mu_b[:, rbase + lo:rbase + lo + NCH])

            for _rep in range(reps):
                # ===== phase A: LN1 + QKV(self half) + pair exchange + scores =====
                with tc.tile_pool(name="ab", bufs=1) as abp:
                    qT = abp.tile([P, CK, TQ], F32R)
                    sums_ps = [pst(1) for _ in range(TQ // NCH)]
                    with tc.tile_pool(name="pa", bufs=1) as pa:
                        r_b = gp.tile([P, TQ], F32, tag="rb")
                        mu_b = gp.tile([P, TQ], F32, tag="murb")
                        r_col = gp.tile([P, TQ // P], F32, tag="rcol")
                        xt = pa.tile([P, CK, TQ], F32R, tag="xt", bufs=1)
                        for k in range(CK):
                            nc.sync.dma_start(xt[:, k, :], xT3[:, k, :])
                        ln_center(xt, TQ, r_b, mu_b, 0, xt, r_col)
                        xh = xt
                        # k projection (self) -> kv_self[0]
                        for j in range(CK):
                            wblk = pa.tile([P, C], F32R, tag="wqk", bufs=3)
                            nc.sync.dma_start(wblk[:], w1qk[CK + j])
                            for sub in range(TQ // NCH):
                                o_ps = pst()
                                for k in range(CK):
                                    nc.tensor.matmul(
                                        o_ps[:], wblk[:, k * P:(k + 1) * P],
                                        xh[:, k, sub * NCH:(sub + 1) * NCH],
                                        start=(k == 0), stop=(k == CK - 1))
                                kv_sb = pa.tile([P, NCH], F32R, tag="vev",
                                                bufs=4)
                                nc.vector.tensor_mul(
                                    kv_sb[:], o_ps[:],
                                    r_b[:, sub * NCH:(sub + 1) * NCH])
                                nc.vector.tensor_scalar(
                                    kv_sb[:], kv_sb[:].bitcast(F32),
                                    c1k_t[:, j:j + 1], None, op0=ALU.add)
                                nc.sync.dma_start(
                                    kv_self[0, j, :,
                                            sub * NCH:(sub + 1) * NCH],
                                    kv_sb[:])
                        # v projection (self, natural layout) -> kv_self[1]
                        for cc in range(C // NCH):
                            w1vh = pa.tile([P, CK, NCH], F32R, tag="w1vh",
                                           bufs=1)
                            for k in range(CK):
                                nc.sync.dma_start(
                                    w1vh[:, k, :],
                                    w1v[k, :, cc * NCH:(cc + 1) * NCH])
                            for sl in range(TQ // P):
                                v_ps = pst()
                                for k in range(CK):
                                    nc.tensor.matmul(
                                        v_ps[:],
                                        xh[:, k, sl * P:(sl + 1) * P],
                                        w1vh[:, k, :],
                                        start=(k == 0), stop=(k == CK - 1))
                                v_sb = pa.tile([P, NCH], F32R, tag="vev",
                                               bufs=4)
                                nc.vector.scalar_tensor_tensor(
                                    v_sb[:], v_ps[:],
                                    r_col[:, sl:sl + 1],
                                    c1v_t[:, cc * NCH:(cc + 1) * NCH],
                                    op0=ALU.mult, op1=ALU.add)
                                nc.sync.dma_start(
                                    kv_self[1, sl, :,
                                            cc * NCH:(cc + 1) * NCH],
                                    v_sb[:])
                        # pair exchange (overlaps with q projection below)
                        nc.gpsimd.collective_compute(
                            "AllGather", mybir.AluOpType.bypass,
                            replica_groups=[[0, 1], [2, 3], [4, 5], [6, 7]],
                            ins=[kv_self[:]], outs=[kv_all[:]])
                        # q projection (self)
                        for j in range(CK):
                            wblk = pa.tile([P, C], F32R, tag="wqk", bufs=3)
                            nc.sync.dma_start(wblk[:], w1qk[j])
                            for sub in range(TQ // NCH):
                                o_ps = pst()
                                for k in range(CK):
                                    nc.tensor.matmul(
                                        o_ps[:], wblk[:, k * P:(k + 1) * P],
                                        xh[:, k, sub * NCH:(sub + 1) * NCH],
                                        start=(k == 0), stop=(k == CK - 1))
                                dst = qT[:, j, sub * NCH:(sub + 1) * NCH]
                                nc.vector.tensor_mul(
                                    dst, o_ps[:],
                                    r_b[:, sub * NCH:(sub + 1) * NCH])
                                nc.vector.tensor_scalar(
                                    dst, dst.bitcast(F32), c1q_t[:, j:j + 1],
                                    None, op0=ALU.add)
                        # attention scores over both gathered halves
                        for g in range(2):
                            for sl in range(TQ // P):
                                s_idx = g * (TQ // P) + sl
                                kt = pa.tile([P, CK, P], F32R, tag="kt",
                                             bufs=3)
                                nc.sync.dma_start(
                                    kt[:],
                                    kv_all[g, 0, :, :, sl * P:(sl + 1) * P]
                                    .rearrange("k p s -> p k s"))
                                for sub in range(TQ // NCH):
                                    a_ps = pst()
                                    for k in range(CK):
                                        nc.tensor.matmul(
                                            a_ps[:], kt[:, k, :],
                                            qT[:, k,
                                               sub * NCH:(sub + 1) * NCH],
                                            start=(k == 0),
                                            stop=(k == CK - 1))
                                    ae = pa.tile([P, NCH], F32R, tag="vev",
                                                 bufs=4)
                                    nc.scalar.activation(ae[:], a_ps[:],
                                                         AF.Exp,
                                                         scale=ATT_SCALE)
                                    nc.sync.dma_start(
                                        attd[s_idx, :,
                                             sub * NCH:(sub + 1) * NCH],
                                        ae[:])
                                    nc.tensor.matmul(
                                        sums_ps[sub][:], ones_col[:], ae[:],
                                        start=(s_idx == 0),
                                        stop=(s_idx == S - 1))

                # ===== phase C: softmax-normalized y, proj, residual =====
                with tc.tile_pool(name="cd", bufs=1) as cdp:
                    x2T = cdp.tile([P, CK, TQ], F32R)
                    with tc.tile_pool(name="pc", bufs=1) as pc:
                        wp_sb = pc.tile([P, CK, C], F32R, tag="wpb")
                        for sub in range(TQ // NCH):
                            srow = pc.tile([1, NCH], F32, tag="srow",
                                           bufs=2)
                            nc.scalar.activation(srow[:], sums_ps[sub][:],
                                                 AF.Copy)
                            nc.vector.reciprocal(srow[:], srow[:])
                            srow_r = pc.tile([1, NCH], F32R, tag="srowr",
                                             bufs=2)
                            nc.scalar.activation(srow_r[:], srow[:],
                                                 AF.Copy)
                            rb_ps = pst()
                            nc.tensor.matmul(rb_ps[:], ones_row[:],
                                             srow_r[:], start=True,
                                             stop=True)
                            nc.vector.tensor_copy(
                                recip_b[:, sub * NCH:(sub + 1) * NCH],
                                rb_ps[:])
                        yT = pc.tile([P, CK, TQ], F32R, tag="yT")
                        for sub in range(TQ // NCH):
                            y_ps = [pst() for _ in range(CK)]
                            for s in range(S):
                                ar = pc.tile([P, NCH], F32R, tag="ar",
                                             bufs=3)
                                nc.sync.dma_start(
                                    ar[:],
                                    attd[s, :, sub * NCH:(sub + 1) * NCH])
                                va = pc.tile([P, C], F32R, tag="va", bufs=3)
                                nc.sync.dma_start(
                                    va[:], kv_all[s // (TQ // P), 1,
                                                  s % (TQ // P)])
                                if sub == 0 and s < CK:
                                    nc.sync.dma_start(wp_sb[:, s, :], wp[s])
                                for cti in range(CK):
                                    nc.tensor.matmul(
                                        y_ps[cti][:],
                                        va[:, cti * P:(cti + 1) * P],
                                        ar[:], start=(s == 0),
                                        stop=(s == S - 1))
                            for cti in range(CK):
                                nc.vector.tensor_mul(
                                    yT[:, cti, sub * NCH:(sub + 1) * NCH],
                                    y_ps[cti][:],
                                    recip_b[:, sub * NCH:(sub + 1) * NCH])
                        for sub in range(TQ // NCH):
                            for j in range(CK):
                                wpb = wp_sb[:, j]
                                z_ps = pst()
                                for k in range(CK):
                                    nc.tensor.matmul(
                                        z_ps[:], wpb[:, k * P:(k + 1) * P],
                                        yT[:, k, sub * NCH:(sub + 1) * NCH],
                                        start=(k == 0), stop=(k == CK - 1))
                                xq = pc.tile([P, NCH], F32R, tag="xq",
                                             bufs=3)
                                nc.sync.dma_start(
                                    xq[:],
                                    xT3[:, j, sub * NCH:(sub + 1) * NCH])
                                nc.vector.scalar_tensor_tensor(
                                    x2T[:, j, sub * NCH:(sub + 1) * NCH],
                                    z_ps[:], bp_t[:, j:j + 1],
                                    xq[:].bitcast(F32),
                                    op0=ALU.add, op1=ALU.add)
                    if debug_taps:
                        nc.sync.dma_start(yT_d[:], yT[:].bitcast(F32))
                        nc.sync.dma_start(x2_d[:], x2T[:].bitcast(F32))
                    # ===== phase D: LN2 + MLP + final residual =====
                    with tc.tile_pool(name="pd", bufs=1) as pd:
                        r2_b = gp.tile([P, TQ], F32, tag="rb")
                        mu2_b = gp.tile([P, TQ], F32, tag="murb")
                        xh2 = pd.tile([P, CK, TQ], F32R, tag="xh2")
                        ln_center(x2T, TQ, r2_b, mu2_b, 0, xh2)
                        if debug_taps:
                            nc.sync.dma_start(xh2_d[:], xh2[:].bitcast(F32))
                        for hhalf in range(2):
                            gel = pd.tile([P, HK // 2, TQ], F32R,
                                          tag="gel", bufs=1)
                            for jl in range(HK // 2):
                                jh = hhalf * (HK // 2) + jl
                                wb2 = pd.tile([P, C], F32R, tag="wb2",
                                              bufs=2)
                                nc.sync.dma_start(wb2[:], w2[jh])
                                for sub in range(TQ // NCH):
                                    m_ps = pst()
                                    for k in range(CK):
                                        nc.tensor.matmul(
                                            m_ps[:],
                                            wb2[:, k * P:(k + 1) * P],
                                            xh2[:, k,
                                                sub * NCH:(sub + 1) * NCH],
                                            start=(k == 0),
                                            stop=(k == CK - 1))
                                    m1t = pd.tile([P, NCH], F32,
                                                  tag="oev", bufs=3)
                                    nc.vector.tensor_mul(
                                        m1t[:], m_ps[:],
                                        r2_b[:, sub * NCH:(sub + 1) * NCH])
                                    nc.scalar.activation(
                                        gel[:, jl,
                                            sub * NCH:(sub + 1) * NCH],
                                        m1t[:], AF.Gelu_apprx_tanh,
                                        bias=c2_t[:, jh:jh + 1])
                            if debug_taps and hhalf == 0:
                                nc.sync.dma_start(gel_d[:], gel[:].bitcast(F32))
                            for j in range(CK):
                                wmh = pd.tile([P, H // 2], F32R, tag="wmh",
                                              bufs=2)
                                nc.sync.dma_start(
                                    wmh[:],
                                    wm[j, :, hhalf * (H // 2):
                                       (hhalf + 1) * (H // 2)])
                                for sub in range(TQ // NCH):
                                    o_ps = pst()
                                    for kk in range(HK // 2):
                                        nc.tensor.matmul(
                                            o_ps[:],
                                            wmh[:, kk * P:(kk + 1) * P],
                                            gel[:, kk,
                                                sub * NCH:(sub + 1) * NCH],
                                            start=(kk == 0),
                                            stop=(kk == HK // 2 - 1))
                                    o_sb = pd.tile([P, NCH], F32,
                                                   tag="oev", bufs=3)
                                    dst = out_t[j * P:(j + 1) * P,
                                                sub * NCH:(sub + 1) * NCH]
                                    if hhalf == 0:
                                        nc.vector.scalar_tensor_tensor(
                                            o_sb[:], o_ps[:],
                                            bm_t[:, j:j + 1],
                                            x2T[:, j,
                                                sub * NCH:(sub + 1) * NCH]
                                            .bitcast(F32),
                                            op0=ALU.add, op1=ALU.add)
                                        nc.sync.dma_start(dst, o_sb[:])
                                    else:
                                        nc.vector.tensor_copy(o_sb[:],
                                                              o_ps[:])
                                        nc.gpsimd.dma_start(
                                            dst, o_sb[:],
                                            accum_op=ALU.add)
    nc.finalize()
    return nc


_prog = None


def _get_prog():
    global _prog
    if _prog is None:
        _prog = _build()
    return _prog


def _pack_weights(ln1_g, ln1_b, w_attn, b_attn, w_proj, b_proj,
                  ln2_g, ln2_b, w_fc, b_fc, w_mlp_proj, b_mlp_proj):
    f = np.float32
    W1 = (ln1_g[:, None] * w_attn).astype(f)            # [C, 3C]
    c1 = (ln1_b @ w_attn + b_attn).astype(f)            # [3C]
    w1qk = np.ascontiguousarray(
        W1[:, :2 * C].reshape(CK, P, 2 * CK, P).transpose(2, 1, 0, 3)
        .reshape(2 * CK, P, C))
    w1v = np.ascontiguousarray(W1[:, 2 * C:].reshape(CK, P, C))
    wp_t = np.ascontiguousarray(
        w_proj.astype(f).reshape(CK, P, CK, P).transpose(2, 1, 0, 3)
        .reshape(CK, P, C))
    W2 = (ln2_g[:, None] * w_fc).astype(f)              # [C, H]
    c2v = (ln2_b @ w_fc + b_fc).astype(f)               # [H]
    w2_t = np.ascontiguousarray(
        W2.reshape(CK, P, HK, P).transpose(2, 1, 0, 3).reshape(HK, P, C))
    wm_t = np.ascontiguousarray(
        w_mlp_proj.astype(f).reshape(HK, P, CK, P).transpose(2, 1, 0, 3)
        .reshape(CK, P, H))
    return {
        "w1qk": w1qk,
        "w1v": w1v,
        "wp": wp_t,
        "w2": w2_t,
        "wm": wm_t,
        "c1q": np.ascontiguousarray(c1[:C].reshape(CK, P)),
        "c1k": np.ascontiguousarray(c1[C:2 * C].reshape(CK, P)),
        "c1vb": np.ascontiguousarray(
            np.broadcast_to(c1[2 * C:], (P, C)).astype(f)),
        "bp": np.ascontiguousarray(b_proj.astype(f).reshape(CK, P)),
        "c2": np.ascontiguousarray(c2v.reshape(HK, P)),
        "bm": np.ascontiguousarray(b_mlp_proj.astype(f).reshape(CK, P)),
        "onc": np.ones((P, 1), f),
        "onr": np.ones((1, P), f),
    }


def kernel(x, ln1_g, ln1_b, w_attn, b_attn, w_proj, b_proj,
           ln2_g, ln2_b, w_fc, b_fc, w_mlp_proj, b_mlp_proj,
           _trace=False):
    x = np.asarray(x, np.float32)
    shared = _pack_weights(
        np.asarray(ln1_g, np.float32), np.asarray(ln1_b, np.float32),
        np.asarray(w_attn, np.float32), np.asarray(b_attn, np.float32),
        np.asarray(w_proj, np.float32), np.asarray(b_proj, np.float32),
        np.asarray(ln2_g, np.float32), np.asarray(ln2_b, np.float32),
        np.asarray(w_fc, np.float32), np.asarray(b_fc, np.float32),
        np.asarray(w_mlp_proj, np.float32), np.asarray(b_mlp_proj, np.float32))

    in_maps = []
    for core in range(N_CORES):
        b, h = core // 2, core % 2
        xTc = np.ascontiguousarray(x[b, h * TQ:(h + 1) * TQ].T)  # [C, TQ]
        in_maps.append({"xT": xTc, **shared})

    nc = _get_prog()
    res = run_bass_kernel_spmd(nc, in_maps, list(range(N_CORES)),
                               trace=_trace)
    out = np.empty_like(x)
    for core in range(N_CORES):
        b, h = core // 2, core % 2
        out[b, h * TQ:(h + 1) * TQ] = res.results[core]["out_t"].T
    if _trace:
        kernel._last_exec_time_ns = res.exec_time_ns
        kernel._last_profile = res.profile_json
        if res.instructions_and_trace is not None:
            kernel._last_trace_path = res.instructions_and_trace[1]
    return out



# revision 3
# speedup vs baseline: 1.4979x; 1.4979x over previous
"""Trainium2 Bass kernel for a dense transformer block (single-head attn + MLP).

Sharding: 8 cores; core c handles batch b=c//2, query-half h=c%2 (1024 queries).
K/V for the peer half arrive via four small bf16 AllGathers (k and v, each
split into two token-halves) kicked as soon as each projection chunk is done,
so the wire time hides under the q-projection and self-half attention scores.
Key order differs per core but softmax over keys is order-invariant.

All matmul operands are bf16 (same PE rate as fp32r at N>=512, half the DMA
and collective bytes, and FWL-accelerated weight loads).  LayerNorm output is
pre-scaled (h = x*r - mu*r) so every projection eviction is a single fused
DVE op.  exp(att) stays in SBUF (no DRAM spill).  Softmax runs without max
subtraction (|logit/32| < 3 for this problem's scale-0.02 weights).
"""

import numpy as np
import ml_dtypes
import concourse.bass as bass
import concourse.mybir as mybir
import concourse.tile as tile
from concourse import bacc
from concourse.bass_utils import run_bass_kernel_spmd

F32 = mybir.dt.float32
F32R = mybir.dt.float32r
BF16 = mybir.dt.bfloat16
AF = mybir.ActivationFunctionType
ALU = mybir.AluOpType

P = 128
C = 1024        # n_embd
T = 2048        # keys per batch
TQ = 1024       # queries per core
H = 4096        # mlp hidden
CK = C // P     # 8
HK = H // P     # 32
S = T // P      # 16 key slices
NCH = 512       # matmul moving-dim chunk
EPS = 1e-5
ATT_SCALE = 1.0 / 32.0   # 1/sqrt(C)

N_CORES = 8
GROUPS = [[0, 1], [2, 3], [4, 5], [6, 7]]


def _build():
    nc = bacc.Bacc()

    xbf = nc.declare_dram_parameter("xbf", [P, CK, TQ], BF16, isOutput=False)
    xf32 = nc.declare_dram_parameter("xf32", [P, CK, TQ], F32, isOutput=False)
    w1kq = nc.declare_dram_parameter("w1kq", [P, 2 * CK, C], BF16,
                                     isOutput=False)
    w1v = nc.declare_dram_parameter("w1v", [P, CK, C], BF16, isOutput=False)
    wp = nc.declare_dram_parameter("wp", [P, CK, C], BF16, isOutput=False)
    w2 = nc.declare_dram_parameter("w2", [HK, P, C], BF16, isOutput=False)
    wm = nc.declare_dram_parameter("wm", [CK, P, H], BF16, isOutput=False)
    c1k = nc.declare_dram_parameter("c1k", [P, CK], F32, isOutput=False)
    c1q = nc.declare_dram_parameter("c1q", [P, CK], F32, isOutput=False)
    c1vb = nc.declare_dram_parameter("c1vb", [P, C], F32, isOutput=False)
    bp = nc.declare_dram_parameter("bp", [P, CK], F32, isOutput=False)
    c2 = nc.declare_dram_parameter("c2", [P, HK], F32, isOutput=False)
    bm = nc.declare_dram_parameter("bm", [P, CK], F32, isOutput=False)
    onc_b = nc.declare_dram_parameter("onc_b", [P, 1], BF16, isOutput=False)
    onc_r = nc.declare_dram_parameter("onc_r", [P, 1], F32R, isOutput=False)
    onr_r = nc.declare_dram_parameter("onr_r", [1, P], F32R, isOutput=False)
    out_t = nc.declare_dram_parameter("out_t", [C, TQ], F32, isOutput=True)

    # exchange buffers: k in [chan, block, tok] layout, v in [tok, chan]
    kv_k = [nc.dram_tensor(f"kv_k{i}", [P, CK, NCH], BF16) for i in range(2)]
    kv_k_all = [nc.dram_tensor(f"kv_k{i}_all", [2, P, CK, NCH], BF16)
                for i in range(2)]
    kv_v = [nc.dram_tensor(f"kv_v{i}", [4, P, C], BF16) for i in range(2)]
    kv_v_all = [nc.dram_tensor(f"kv_v{i}_all", [2, 4, P, C], BF16)
                for i in range(2)]

    with tile.TileContext(nc) as tc:
        with (
            tc.tile_pool(name="glob", bufs=1) as gp,
            tc.tile_pool(name="ps", bufs=8, space="PSUM") as pp,
        ):
            def pst(pdim=P):
                return pp.tile([pdim, NCH], F32, tag="ps", name="ps")

            ones_b = gp.tile([P, 1], BF16)
            nc.sync.dma_start(ones_b[:], onc_b[:])
            ones_r = gp.tile([P, 1], F32R)
            nc.sync.dma_start(ones_r[:], onc_r[:])
            ones_row = gp.tile([1, P], F32R)
            nc.sync.dma_start(ones_row[:], onr_r[:])
            c1k_t = gp.tile([P, CK], F32)
            nc.sync.dma_start(c1k_t[:], c1k[:])
            c1q_t = gp.tile([P, CK], F32)
            nc.sync.dma_start(c1q_t[:], c1q[:])
            c1v_t = gp.tile([P, C], F32)
            nc.sync.dma_start(c1v_t[:], c1vb[:])
            bp_t = gp.tile([P, CK], F32)
            nc.sync.dma_start(bp_t[:], bp[:])
            c2_t = gp.tile([P, HK], F32)
            nc.sync.dma_start(c2_t[:], c2[:])
            bm_t = gp.tile([P, CK], F32)
            nc.sync.dma_start(bm_t[:], bm[:])
            eps_col = gp.tile([P, 1], F32)
            nc.vector.memset(eps_col[:], EPS)

            qT = gp.tile([P, CK, TQ], BF16, tag="qT")
            x2T = gp.tile([P, CK, TQ], F32R, tag="x2T")
            recip_b = gp.tile([P, TQ], F32, tag="recip")

            def ln_scale(pool, src3, ones_col, is_bf, r_b, mur_b, dst3):
                """LN stats over channels (partition dim) of [P, CK, TQ]
                activations; writes dst3 = (src - mu) * rsigma in bf16.
                src3 may equal dst3 (in-place)."""
                for sub in range(TQ // NCH):
                    lo = sub * NCH
                    mu_ps = pst(1)
                    s2_ps = pst(1)
                    for k in range(CK):
                        nc.tensor.matmul(mu_ps[:], ones_col[:],
                                         src3[:, k, lo:lo + NCH],
                                         start=(k == 0), stop=(k == CK - 1))
                    for k in range(CK):
                        sq = pool.tile([P, NCH], BF16 if is_bf else F32R,
                                       tag="sq", bufs=2)
                        src = src3[:, k, lo:lo + NCH]
                        nc.scalar.activation(
                            sq[:], src if is_bf else src.bitcast(F32),
                            AF.Square)
                        nc.tensor.matmul(s2_ps[:], ones_col[:], sq[:],
                                         start=(k == 0), stop=(k == CK - 1))
                    mu_row = pool.tile([1, NCH], F32, tag="murow", bufs=2)
                    nc.scalar.activation(mu_row[:], mu_ps[:], AF.Copy,
                                         scale=1.0 / C)
                    musq = pool.tile([1, NCH], F32, tag="musq", bufs=2)
                    nc.scalar.activation(musq[:], mu_ps[:], AF.Square,
                                         scale=1.0 / C)
                    sig = pool.tile([1, NCH], F32, tag="sig", bufs=2)
                    nc.vector.scalar_tensor_tensor(
                        sig[:], s2_ps[:], 1.0 / C, musq[:],
                        op0=ALU.mult, op1=ALU.subtract)
                    nc.scalar.activation(sig[:], sig[:], AF.Sqrt,
                                         bias=eps_col[0:1])
                    nc.vector.reciprocal(sig[:], sig[:])
                    rrow = pool.tile([1, NCH], F32R, tag="rrow", bufs=2)
                    nc.scalar.activation(rrow[:], sig[:], AF.Copy)
                    mrrow = pool.tile([1, NCH], F32R, tag="mrrow", bufs=2)
                    nc.vector.tensor_mul(mrrow[:], mu_row[:], sig[:])
                    rb_ps = pst()
                    nc.tensor.matmul(rb_ps[:], ones_row[:], rrow[:],
                                     start=True, stop=True)
                    nc.vector.tensor_copy(r_b[:, lo:lo + NCH], rb_ps[:])
                    mr_ps = pst()
                    nc.tensor.matmul(mr_ps[:], ones_row[:], mrrow[:],
                                     start=True, stop=True)
                    nc.vector.tensor_copy(mur_b[:, lo:lo + NCH], mr_ps[:])
                    for k in range(CK):
                        tmp = pool.tile([P, NCH], F32, tag="htmp", bufs=3)
                        src = src3[:, k, lo:lo + NCH]
                        nc.vector.tensor_mul(
                            tmp[:], src if is_bf else src.bitcast(F32),
                            r_b[:, lo:lo + NCH])
                        nc.vector.tensor_sub(dst3[:, k, lo:lo + NCH],
                                             tmp[:], mur_b[:, lo:lo + NCH])

            # ===== phase A: LN1, qkv projections, kv pair exchange =====
            with tc.tile_pool(name="pa", bufs=1) as pa:
                xt = pa.tile([P, CK, TQ], BF16, tag="xt")
                nc.sync.dma_start(xt[:], xbf[:])
                w1k_sb = pa.tile([P, CK, C], BF16, tag="w1k")
                nc.sync.dma_start(w1k_sb[:], w1kq[:, 0:CK, :])
                w1v_sb = pa.tile([P, CK, C], BF16, tag="w1v")
                nc.sync.dma_start(w1v_sb[:], w1v[:])
                w1q_sb = pa.tile([P, CK, C], BF16, tag="w1q")
                nc.sync.dma_start(w1q_sb[:], w1kq[:, CK:2 * CK, :])

                r_b = pa.tile([P, TQ], F32, tag="rb")
                mur_b = pa.tile([P, TQ], F32, tag="murb")
                ln_scale(pa, xt, ones_b, True, r_b, mur_b, xt)

                # k projection; exchange each token-half as soon as ready
                for sub in range(2):
                    lo = sub * NCH
                    for jo in range(CK):
                        o_ps = pst()
                        for ki in range(CK):
                            nc.tensor.matmul(
                                o_ps[:],
                                w1k_sb[:, jo, ki * P:(ki + 1) * P],
                                xt[:, ki, lo:lo + NCH],
                                start=(ki == 0), stop=(ki == CK - 1))
                        ev = pa.tile([P, NCH], BF16, tag="vev", bufs=4)
                        nc.vector.tensor_scalar(
                            ev[:], o_ps[:], c1k_t[:, jo:jo + 1], None,
                            op0=ALU.add)
                        nc.sync.dma_start(kv_k[sub][:, jo, :], ev[:])
                    nc.gpsimd.collective_compute(
                        "AllGather", ALU.bypass, replica_groups=GROUPS,
                        ins=[kv_k[sub][:]], outs=[kv_k_all[sub][:]])

                # v projection (natural layout), exchange per token-half
                for th in range(2):
                    for sl in range(4):
                        sa = th * 4 + sl
                        for cc in range(2):
                            v_ps = pst()
                            for ki in range(CK):
                                nc.tensor.matmul(
                                    v_ps[:],
                                    xt[:, ki, sa * P:(sa + 1) * P],
                                    w1v_sb[:, ki, cc * NCH:(cc + 1) * NCH],
                                    start=(ki == 0), stop=(ki == CK - 1))
                            ev = pa.tile([P, NCH], BF16, tag="vev", bufs=4)
                            nc.vector.tensor_add(
                                ev[:], v_ps[:],
                                c1v_t[:, cc * NCH:(cc + 1) * NCH])
                            nc.sync.dma_start(
                                kv_v[th][sl, :, cc * NCH:(cc + 1) * NCH],
                                ev[:])
                    nc.gpsimd.collective_compute(
                        "AllGather", ALU.bypass, replica_groups=GROUPS,
                        ins=[kv_v[th][:]], outs=[kv_v_all[th][:]])

                # q projection
                for sub in range(2):
                    lo = sub * NCH
                    for jo in range(CK):
                        o_ps = pst()
                        for ki in range(CK):
                            nc.tensor.matmul(
                                o_ps[:],
                                w1q_sb[:, jo, ki * P:(ki + 1) * P],
                                xt[:, ki, lo:lo + NCH],
                                start=(ki == 0), stop=(ki == CK - 1))
                        nc.vector.tensor_scalar(
                            qT[:, jo, lo:lo + NCH], o_ps[:],
                            c1q_t[:, jo:jo + 1], None, op0=ALU.add)

            # ===== phase B: scores + softmax numerator/denominator =====
            # ===== phase C: y = att @ v, proj, residual =====
            with tc.tile_pool(name="pb", bufs=1) as pb:
                att = pb.tile([P, S, TQ], BF16, tag="att")
                sums_ps = [pst(1) for _ in range(2)]
                for th in range(2):
                    for g in range(2):
                        kts = pb.tile([P, CK, NCH], BF16, tag="kt", bufs=2)
                        nc.sync.dma_start(kts[:], kv_k_all[th][g])
                        for sl in range(4):
                            s_idx = g * 8 + th * 4 + sl
                            order = th * 8 + g * 4 + sl
                            for sub in range(2):
                                lo = sub * NCH
                                a_ps = pst()
                                for ki in range(CK):
                                    nc.tensor.matmul(
                                        a_ps[:],
                                        kts[:, ki, sl * P:(sl + 1) * P],
                                        qT[:, ki, lo:lo + NCH],
                                        start=(ki == 0), stop=(ki == CK - 1))
                                nc.scalar.activation(
                                    att[:, s_idx, lo:lo + NCH], a_ps[:],
                                    AF.Exp, scale=ATT_SCALE)
                                nc.tensor.matmul(
                                    sums_ps[sub][:], ones_b[:],
                                    att[:, s_idx, lo:lo + NCH],
                                    start=(order == 0), stop=(order == S - 1))
                # softmax denominators -> broadcast reciprocal
                for sub in range(2):
                    lo = sub * NCH
                    srow = pb.tile([1, NCH], F32, tag="srow", bufs=2)
                    nc.scalar.activation(srow[:], sums_ps[sub][:], AF.Copy)
                    nc.vector.reciprocal(srow[:], srow[:])
                    srow_r = pb.tile([1, NCH], F32R, tag="srowr", bufs=2)
                    nc.scalar.activation(srow_r[:], srow[:], AF.Copy)
                    rb_ps = pst()
                    nc.tensor.matmul(rb_ps[:], ones_row[:], srow_r[:],
                                     start=True, stop=True)
                    nc.vector.tensor_copy(recip_b[:, lo:lo + NCH], rb_ps[:])

                v_sb = pb.tile([P, S, C], BF16, tag="vsb")
                for th in range(2):
                    for g in range(2):
                        for sl in range(4):
                            nc.sync.dma_start(
                                v_sb[:, g * 8 + th * 4 + sl, :],
                                kv_v_all[th][g, sl])
                wp_sb = pb.tile([P, CK, C], BF16, tag="wpb")
                nc.sync.dma_start(wp_sb[:], wp[:])

                yT = pb.tile([P, CK, TQ], BF16, tag="yT")
                for sub in range(2):
                    lo = sub * NCH
                    y_ps = [pst() for _ in range(CK)]
                    for s in range(S):
                        for cti in range(CK):
                            nc.tensor.matmul(
                                y_ps[cti][:],
                                v_sb[:, s, cti * P:(cti + 1) * P],
                                att[:, s, lo:lo + NCH],
                                start=(s == 0), stop=(s == S - 1))
                    for cti in range(CK):
                        nc.vector.tensor_mul(
                            yT[:, cti, lo:lo + NCH], y_ps[cti][:],
                            recip_b[:, lo:lo + NCH])
                # proj + residual
                for sub in range(2):
                    lo = sub * NCH
                    for jo in range(CK):
                        z_ps = pst()
                        for ki in range(CK):
                            nc.tensor.matmul(
                                z_ps[:], wp_sb[:, jo, ki * P:(ki + 1) * P],
                                yT[:, ki, lo:lo + NCH],
                                start=(ki == 0), stop=(ki == CK - 1))
                        xq = pb.tile([P, NCH], F32, tag="xq", bufs=3)
                        nc.sync.dma_start(xq[:], xf32[:, jo, lo:lo + NCH])
                        nc.vector.scalar_tensor_tensor(
                            x2T[:, jo, lo:lo + NCH], z_ps[:],
                            bp_t[:, jo:jo + 1], xq[:],
                            op0=ALU.add, op1=ALU.add)

            # ===== phase D: LN2 + MLP + final residual =====
            with tc.tile_pool(name="pd", bufs=1) as pd:
                r2_b = pd.tile([P, TQ], F32, tag="rb")
                mur2_b = pd.tile([P, TQ], F32, tag="murb")
                xh2 = pd.tile([P, CK, TQ], BF16, tag="xh2")
                ln_scale(pd, x2T, ones_r, False, r2_b, mur2_b, xh2)
                for hhalf in range(2):
                    gel = pd.tile([P, HK // 2, TQ], BF16, tag="gel", bufs=2)
                    for jl in range(HK // 2):
                        jh = hhalf * (HK // 2) + jl
                        w2_sb = pd.tile([P, C], BF16, tag="w2", bufs=3)
                        nc.sync.dma_start(w2_sb[:], w2[jh])
                        for sub in range(2):
                            lo = sub * NCH
                            m_ps = pst()
                            for ki in range(CK):
                                nc.tensor.matmul(
                                    m_ps[:], w2_sb[:, ki * P:(ki + 1) * P],
                                    xh2[:, ki, lo:lo + NCH],
                                    start=(ki == 0), stop=(ki == CK - 1))
                            nc.scalar.activation(
                                gel[:, jl, lo:lo + NCH], m_ps[:],
                                AF.Gelu_apprx_tanh, bias=c2_t[:, jh:jh + 1])
                    for jo in range(CK):
                        wm_sb = pd.tile([P, H // 2], BF16, tag="wm", bufs=2)
                        nc.sync.dma_start(
                            wm_sb[:],
                            wm[jo, :, hhalf * (H // 2):(hhalf + 1) * (H // 2)])
                        for sub in range(2):
                            lo = sub * NCH
                            o_ps = pst()
                            for kk in range(HK // 2):
                                nc.tensor.matmul(
                                    o_ps[:], wm_sb[:, kk * P:(kk + 1) * P],
                                    gel[:, kk, lo:lo + NCH],
                                    start=(kk == 0), stop=(kk == HK // 2 - 1))
                            o_sb = pd.tile([P, NCH], F32, tag="oev", bufs=4)
                            dst = out_t[jo * P:(jo + 1) * P, lo:lo + NCH]
                            if hhalf == 0:
                                nc.vector.scalar_tensor_tensor(
                                    o_sb[:], o_ps[:], bm_t[:, jo:jo + 1],
                                    x2T[:, jo, lo:lo + NCH].bitcast(F32),
                                    op0=ALU.add, op1=ALU.add)
                                nc.sync.dma_start(dst, o_sb[:])
                            else:
                                nc.vector.tensor_copy(o_sb[:], o_ps[:])
                                nc.gpsimd.dma_start(dst, o_sb[:],
                                                    accum_op=ALU.add)
    nc.finalize()
    return nc


_prog = None


def _get_prog():
    global _prog
    if _prog is None:
        _prog = _build()
    return _prog


def _pack_weights(ln1_g, ln1_b, w_attn, b_attn, w_proj, b_proj,
                  ln2_g, ln2_b, w_fc, b_fc, w_mlp_proj, b_mlp_proj):
    f = np.float32
    bf = ml_dtypes.bfloat16
    W1 = (ln1_g[:, None] * w_attn).astype(f)            # [C, 3C]
    c1 = (ln1_b @ w_attn + b_attn).astype(f)            # [3C]
    kq = np.concatenate([W1[:, C:2 * C], W1[:, :C]], axis=1)  # k then q
    w1kq_h = np.ascontiguousarray(
        kq.reshape(CK, P, 2 * CK, P).transpose(1, 2, 0, 3)
        .reshape(P, 2 * CK, C)).astype(bf)
    w1v_h = np.ascontiguousarray(
        W1[:, 2 * C:].reshape(CK, P, C).transpose(1, 0, 2)).astype(bf)
    wp_h = np.ascontiguousarray(
        w_proj.astype(f).reshape(CK, P, CK, P).transpose(1, 2, 0, 3)
        .reshape(P, CK, C)).astype(bf)
    W2 = (ln2_g[:, None] * w_fc).astype(f)              # [C, H]
    c2v = (ln2_b @ w_fc + b_fc).astype(f)               # [H]
    w2_h = np.ascontiguousarray(
        W2.reshape(CK, P, HK, P).transpose(2, 1, 0, 3).reshape(HK, P, C)
    ).astype(bf)
    wm_h = np.ascontiguousarray(
        w_mlp_proj.astype(f).reshape(HK, P, CK, P).transpose(2, 1, 0, 3)
        .reshape(CK, P, H)).astype(bf)
    return {
        "w1kq": w1kq_h,
        "w1v": w1v_h,
        "wp": wp_h,
        "w2": w2_h,
        "wm": wm_h,
        "c1k": np.ascontiguousarray(c1[C:2 * C].reshape(CK, P).T).astype(f),
        "c1q": np.ascontiguousarray(c1[:C].reshape(CK, P).T).astype(f),
        "c1vb": np.ascontiguousarray(
            np.broadcast_to(c1[2 * C:], (P, C))).astype(f),
        "bp": np.ascontiguousarray(
            b_proj.astype(f).reshape(CK, P).T).astype(f),
        "c2": np.ascontiguousarray(c2v.reshape(HK, P).T).astype(f),
        "bm": np.ascontiguousarray(
            b_mlp_proj.astype(f).reshape(CK, P).T).astype(f),
        "onc_b": np.ones((P, 1), ml_dtypes.bfloat16),
        "onc_r": np.ones((P, 1), f),
        "onr_r": np.ones((1, P), f),
    }


def kernel(x, ln1_g, ln1_b, w_attn, b_attn, w_proj, b_proj,
           ln2_g, ln2_b, w_fc, b_fc, w_mlp_proj, b_mlp_proj,
           _trace=False):
    x = np.asarray(x, np.float32)
    shared = _pack_weights(
        np.asarray(ln1_g, np.float32), np.asarray(ln1_b, np.float32),
        np.asarray(w_attn, np.float32), np.asarray(b_attn, np.float32),
        np.asarray(w_proj, np.float32), np.asarray(b_proj, np.float32),
        np.asarray(ln2_g, np.float32), np.asarray(ln2_b, np.float32),
        np.asarray(w_fc, np.float32), np.asarray(b_fc, np.float32),
        np.asarray(w_mlp_proj, np.float32),
        np.asarray(b_mlp_proj, np.float32))

    in_maps = []
    for core in range(N_CORES):
        b, h = core // 2, core % 2
        xTc = np.ascontiguousarray(
            x[b, h * TQ:(h + 1) * TQ].T.reshape(CK, P, TQ)
            .transpose(1, 0, 2))                         # [P, CK, TQ]
        in_maps.append({
            "xbf": xTc.astype(ml_dtypes.bfloat16),
            "xf32": xTc.astype(np.float32),
            **shared,
        })

    nc = _get_prog()
    res = run_bass_kernel_spmd(nc, in_maps, list(range(N_CORES)),
                               trace=_trace)
    out = np.empty_like(x)
    for core in range(N_CORES):
        b, h = core // 2, core % 2
        out[b, h * TQ:(h + 1) * TQ] = res.results[core]["out_t"].T
    if _trace:
        kernel._last_exec_time_ns = res.exec_time_ns
        kernel._last_profile = res.profile_json
        if res.instructions_and_trace is not None:
            kernel._last_trace_path = res.instructions_and_trace[1]
    return out


# revision 11
# speedup vs baseline: 1.5904x; 1.0618x over previous
"""Trainium2 Bass kernel for a dense transformer block (single-head attn + MLP).

Sharding: 8 cores; core c handles batch b=c//2, query-half h=c%2 (1024 queries).
K/V for the peer half arrive via four small bf16 AllGathers (k and v, each
split into two token-halves) kicked as soon as each projection chunk is done;
a tiny dummy AllGather at kernel start absorbs the first-collective setup
cost.  Key order differs per core but softmax over keys is order-invariant.

All matmul operands are bf16.  LayerNorm is folded algebraically: projections
run on RAW x (so the PE never waits for LN statistics) and the per-token
scale/shift is applied at PSUM eviction via
    proj(LN(x)) = r * (W.T x) - (r*mu) * colsum(W)   (+ bias)
with colsum(W) precomputed on the host.  exp(att) stays in SBUF (no DRAM
spill).  Softmax runs without max subtraction (|logit/32| < 3 here).  The
residual stream is carried in bf16; the MLP output accumulates across the two
hidden halves in SBUF.  LN2 statistics matmuls are interleaved with the proj
evictions that produce their input so the PE never drains.
"""

import numpy as np
import ml_dtypes
import concourse.bass as bass
import concourse.mybir as mybir
import concourse.tile as tile
from concourse import bacc
from concourse.bass_utils import run_bass_kernel_spmd

F32 = mybir.dt.float32
F32R = mybir.dt.float32r
BF16 = mybir.dt.bfloat16
AF = mybir.ActivationFunctionType
ALU = mybir.AluOpType

P = 128
C = 1024        # n_embd
T = 2048        # keys per batch
TQ = 1024       # queries per core
H = 4096        # mlp hidden
CK = C // P     # 8
HK = H // P     # 32
S = T // P      # 16 key slices
NCH = 512       # matmul moving-dim chunk
EPS = 1e-5
ATT_SCALE = 1.0 / 32.0   # 1/sqrt(C)

N_CORES = 8
GROUPS = [[0, 1], [2, 3], [4, 5], [6, 7]]


def _build(qkv_bias=False):
    nc = bacc.Bacc()

    xbf = nc.declare_dram_parameter("xbf", [2, P, CK, NCH], BF16,
                                    isOutput=False)
    w1kq = nc.declare_dram_parameter("w1kq", [P, 2 * CK, C], BF16,
                                     isOutput=False)
    w1v = nc.declare_dram_parameter("w1v", [P, CK, C], BF16, isOutput=False)
    wp = nc.declare_dram_parameter("wp", [P, CK, C], BF16, isOutput=False)
    w2 = nc.declare_dram_parameter("w2", [HK, P, C], BF16, isOutput=False)
    wm = nc.declare_dram_parameter("wm", [CK, P, H], BF16, isOutput=False)
    # folded LN columns: negated column sums of each weight block
    s1kn = nc.declare_dram_parameter("s1kn", [P, CK], F32, isOutput=False)
    s1qn = nc.declare_dram_parameter("s1qn", [P, CK], F32, isOutput=False)
    s1vnb = nc.declare_dram_parameter("s1vnb", [P, C], F32, isOutput=False)
    s2n = nc.declare_dram_parameter("s2n", [P, HK], F32, isOutput=False)
    c1k = nc.declare_dram_parameter("c1k", [P, CK], F32, isOutput=False)
    c1q = nc.declare_dram_parameter("c1q", [P, CK], F32, isOutput=False)
    c1vb = nc.declare_dram_parameter("c1vb", [P, C], F32, isOutput=False)
    bp = nc.declare_dram_parameter("bp", [P, CK], F32, isOutput=False)
    c2 = nc.declare_dram_parameter("c2", [P, HK], F32, isOutput=False)
    bm = nc.declare_dram_parameter("bm", [P, CK], F32, isOutput=False)
    onc_b = nc.declare_dram_parameter("onc_b", [P, 1], BF16, isOutput=False)
    onr_r = nc.declare_dram_parameter("onr_r", [1, P], F32R, isOutput=False)
    out_t = nc.declare_dram_parameter("out_t", [C, TQ], F32, isOutput=True)

    # exchange buffers: k in [chan, block, tok] layout, v in [tok, chan]
    warm = nc.dram_tensor("warm", [1, 64], BF16)
    warm_all = nc.dram_tensor("warm_all", [2, 1, 64], BF16)
    kv_k = [nc.dram_tensor(f"kv_k{i}", [P, CK, NCH], BF16) for i in range(2)]
    kv_k_all = [nc.dram_tensor(f"kv_k{i}_all", [2, P, CK, NCH], BF16)
                for i in range(2)]
    kv_v = [nc.dram_tensor(f"kv_v{i}", [4, P, C], BF16) for i in range(2)]
    kv_v_all = [nc.dram_tensor(f"kv_v{i}_all", [2, 4, P, C], BF16)
                for i in range(2)]

    with tile.TileContext(nc) as tc:
        with (
            tc.tile_pool(name="glob", bufs=1) as gp,
            tc.tile_pool(name="ps", bufs=8, space="PSUM") as pp,
        ):
            def pst(pdim=P):
                return pp.tile([pdim, NCH], F32, tag="ps", name="ps")

            # warm up the collective path while inputs stream in
            nc.gpsimd.collective_compute(
                "AllGather", ALU.bypass, replica_groups=GROUPS,
                ins=[warm[:]], outs=[warm_all[:]])

            # critical-path DMAs first: ones (stats lhsT), x, k weights
            ones_b = gp.tile([P, 1], BF16)
            nc.sync.dma_start(ones_b[:], onc_b[:])
            pAB_cm = tc.tile_pool(name="pAB", bufs=1)
            pAB = pAB_cm.__enter__()
            xt = pAB.tile([P, CK, TQ], BF16, tag="xt")
            for sub in range(2):
                nc.sync.dma_start(xt[:, :, sub * NCH:(sub + 1) * NCH],
                                  xbf[sub])

            with tc.tile_pool(name="pa", bufs=1) as pa:
                w1k_sb = pa.tile([P, CK, C], BF16, tag="w1k")
                nc.sync.dma_start(w1k_sb[:], w1kq[:, 0:CK, :])

                ones_row = gp.tile([1, P], F32R)
                nc.sync.dma_start(ones_row[:], onr_r[:])
                s1kn_t = gp.tile([P, CK], F32)
                nc.sync.dma_start(s1kn_t[:], s1kn[:])
                s1qn_t = gp.tile([P, CK], F32)
                nc.sync.dma_start(s1qn_t[:], s1qn[:])
                s1vn_t = gp.tile([P, C], F32)
                nc.sync.dma_start(s1vn_t[:], s1vnb[:])
                s2n_t = gp.tile([P, HK], F32)
                nc.sync.dma_start(s2n_t[:], s2n[:])
                if qkv_bias:
                    c1k_t = gp.tile([P, CK], F32)
                    nc.sync.dma_start(c1k_t[:], c1k[:])
                    c1q_t = gp.tile([P, CK], F32)
                    nc.sync.dma_start(c1q_t[:], c1q[:])
                c1v_t = gp.tile([P, C], F32)
                nc.sync.dma_start(c1v_t[:], c1vb[:])
                bp_t = gp.tile([P, CK], F32)
                nc.sync.dma_start(bp_t[:], bp[:])
                c2_t = gp.tile([P, HK], F32)
                nc.sync.dma_start(c2_t[:], c2[:])
                bm_t = gp.tile([P, CK], F32)
                nc.sync.dma_start(bm_t[:], bm[:])
                eps_col = gp.tile([P, 1], F32)
                nc.vector.memset(eps_col[:], EPS)
                w1v_sb = pa.tile([P, CK, C], BF16, tag="w1v")
                nc.sync.dma_start(w1v_sb[:], w1v[:])
                w1q_sb = pa.tile([P, CK, C], BF16, tag="w1q")
                nc.sync.dma_start(w1q_sb[:], w1kq[:, CK:2 * CK, :])

                def stats_mms(pool, src3, sub, sq_on_dve):
                    """mean and sum-of-squares matmuls for one 512-chunk."""
                    lo = sub * NCH
                    mu_ps = pst(1)
                    s2_ps = pst(1)
                    for k in range(CK):
                        nc.tensor.matmul(mu_ps[:], ones_b[:],
                                         src3[:, k, lo:lo + NCH],
                                         start=(k == 0), stop=(k == CK - 1))
                    for k in range(CK):
                        sq = pool.tile([P, NCH], BF16, tag="sq", bufs=4)
                        src = src3[:, k, lo:lo + NCH]
                        if sq_on_dve:
                            nc.vector.tensor_mul(sq[:], src, src)
                        else:
                            nc.scalar.activation(sq[:], src, AF.Square)
                        nc.tensor.matmul(s2_ps[:], ones_b[:], sq[:],
                                         start=(k == 0), stop=(k == CK - 1))
                    return mu_ps, s2_ps

                def ln_rows(pool, stats):
                    """row chains: per chunk (mu, rsigma, mu*rsigma) rows."""
                    rows = []
                    for mu_ps, s2_ps in stats:
                        mu_row = pool.tile([1, NCH], F32R, tag="murow",
                                           bufs=2)
                        nc.scalar.activation(mu_row[:], mu_ps[:], AF.Copy,
                                             scale=1.0 / C)
                        musq = pool.tile([1, NCH], F32, tag="musq", bufs=2)
                        nc.scalar.activation(musq[:], mu_ps[:], AF.Square,
                                             scale=1.0 / C)
                        sig = pool.tile([1, NCH], F32, tag="sig", bufs=2)
                        nc.vector.scalar_tensor_tensor(
                            sig[:], s2_ps[:], 1.0 / C, musq[:],
                            op0=ALU.mult, op1=ALU.subtract)
                        nc.scalar.activation(sig[:], sig[:], AF.Sqrt,
                                             bias=eps_col[0:1])
                        nc.vector.reciprocal(sig[:], sig[:])
                        rrow = pool.tile([1, NCH], F32R, tag="rrow", bufs=2)
                        nc.vector.tensor_copy(rrow[:], sig[:])
                        mrrow = pool.tile([1, NCH], F32R, tag="mrrow",
                                          bufs=2)
                        nc.vector.tensor_mul(mrrow[:],
                                             mu_row[:].bitcast(F32), sig[:])
                        rows.append((mu_row, rrow, mrrow))
                    return rows

                def ln_bcast(rows, r_b, mur_b):
                    for sub, (_, rrow, mrrow) in enumerate(rows):
                        lo = sub * NCH
                        rb_ps = pst()
                        nc.tensor.matmul(rb_ps[:], ones_row[:], rrow[:],
                                         start=True, stop=True)
                        nc.vector.tensor_copy(r_b[:, lo:lo + NCH], rb_ps[:])
                        mr_ps = pst()
                        nc.tensor.matmul(mr_ps[:], ones_row[:], mrrow[:],
                                         start=True, stop=True)
                        nc.vector.tensor_copy(mur_b[:, lo:lo + NCH],
                                              mr_ps[:])

                # ===== phase A: LN1 stats, qkv on raw x, kv exchange =====
                r_b = pa.tile([P, TQ], F32, tag="rb")
                mur_b = pa.tile([P, TQ], F32, tag="murb")
                r_col = pa.tile([P, TQ // P], F32, tag="rcol")
                mu_col = pa.tile([P, TQ // P], F32, tag="mucol")
                stats = [stats_mms(pa, xt, sub, sq_on_dve=True)
                         for sub in range(2)]
                rows1 = ln_rows(pa, stats)
                ln_bcast(rows1, r_b, mur_b)

                def kq_evict(dst, o_ps, lo, sn_t, c1_t, jo):
                    tmp = pa.tile([P, NCH], F32, tag="ktmp", bufs=3)
                    nc.vector.tensor_mul(tmp[:], o_ps[:], r_b[:, lo:lo + NCH])
                    nc.vector.scalar_tensor_tensor(
                        dst, mur_b[:, lo:lo + NCH], sn_t[:, jo:jo + 1],
                        tmp[:], op0=ALU.mult, op1=ALU.add)
                    if qkv_bias:
                        nc.vector.tensor_scalar(
                            dst, dst, c1_t[:, jo:jo + 1], None, op0=ALU.add)

                # k projection; exchange each token-half as soon as ready
                for sub in range(2):
                    lo = sub * NCH
                    for jo in range(CK):
                        o_ps = pst()
                        for ki in range(CK):
                            nc.tensor.matmul(
                                o_ps[:],
                                w1k_sb[:, jo, ki * P:(ki + 1) * P],
                                xt[:, ki, lo:lo + NCH],
                                start=(ki == 0), stop=(ki == CK - 1))
                        ev = pa.tile([P, NCH], BF16, tag="vev", bufs=4)
                        kq_evict(ev[:], o_ps, lo, s1kn_t,
                                 c1k_t if qkv_bias else None, jo)
                        nc.sync.dma_start(kv_k[sub][:, jo, :], ev[:])
                    nc.gpsimd.collective_compute(
                        "AllGather", ALU.bypass, replica_groups=GROUPS,
                        ins=[kv_k[sub][:]], outs=[kv_k_all[sub][:]])

                # r/mu per-token columns for the v eviction
                for sub in range(2):
                    mu_row, rrow, _ = rows1[sub]
                    for b in range(NCH // P):
                        blk = sub * (NCH // P) + b
                        for row, col in ((rrow, r_col), (mu_row, mu_col)):
                            rc_ps = pst()
                            nc.tensor.matmul(rc_ps[:, 0:P],
                                             row[:, b * P:(b + 1) * P],
                                             ones_row[:],
                                             start=True, stop=True)
                            nc.vector.tensor_copy(col[:, blk:blk + 1],
                                                  rc_ps[:, 0:1])

                # v projection (natural layout), exchange per token-half
                for th in range(2):
                    for sl in range(4):
                        sa = th * 4 + sl
                        for cc in range(2):
                            v_ps = pst()
                            for ki in range(CK):
                                nc.tensor.matmul(
                                    v_ps[:],
                                    xt[:, ki, sa * P:(sa + 1) * P],
                                    w1v_sb[:, ki, cc * NCH:(cc + 1) * NCH],
                                    start=(ki == 0), stop=(ki == CK - 1))
                            t1 = pa.tile([P, NCH], F32, tag="ktmp", bufs=3)
                            nc.vector.scalar_tensor_tensor(
                                t1[:], s1vn_t[:, cc * NCH:(cc + 1) * NCH],
                                mu_col[:, sa:sa + 1], v_ps[:],
                                op0=ALU.mult, op1=ALU.add)
                            ev = pa.tile([P, NCH], BF16, tag="vev", bufs=4)
                            nc.vector.scalar_tensor_tensor(
                                ev[:], t1[:], r_col[:, sa:sa + 1],
                                c1v_t[:, cc * NCH:(cc + 1) * NCH],
                                op0=ALU.mult, op1=ALU.add)
                            nc.sync.dma_start(
                                kv_v[th][sl, :, cc * NCH:(cc + 1) * NCH],
                                ev[:])
                    nc.gpsimd.collective_compute(
                        "AllGather", ALU.bypass, replica_groups=GROUPS,
                        ins=[kv_v[th][:]], outs=[kv_v_all[th][:]])

                # q projection
                qT = pAB.tile([P, CK, TQ], BF16, tag="qT")
                for sub in range(2):
                    lo = sub * NCH
                    for jo in range(CK):
                        o_ps = pst()
                        for ki in range(CK):
                            nc.tensor.matmul(
                                o_ps[:],
                                w1q_sb[:, jo, ki * P:(ki + 1) * P],
                                xt[:, ki, lo:lo + NCH],
                                start=(ki == 0), stop=(ki == CK - 1))
                        kq_evict(qT[:, jo, lo:lo + NCH], o_ps, lo, s1qn_t,
                                 c1q_t if qkv_bias else None, jo)

            # ===== phase B: scores + softmax, then y, proj, residual ====
            with tc.tile_pool(name="pb", bufs=1) as pb:
                att = pb.tile([P, S, TQ], BF16, tag="att")
                recip_b = pb.tile([P, TQ], F32, tag="recip")
                sums_ps = [pst(1) for _ in range(2)]
                for th in range(2):
                    for g in range(2):
                        kts = pb.tile([P, CK, NCH], BF16, tag="kt", bufs=3)
                        nc.sync.dma_start(kts[:], kv_k_all[th][g])
                        for sl in range(4):
                            s_idx = g * 8 + th * 4 + sl
                            order = th * 8 + g * 4 + sl
                            for sub in range(2):
                                lo = sub * NCH
                                a_ps = pst()
                                for ki in range(CK):
                                    nc.tensor.matmul(
                                        a_ps[:],
                                        kts[:, ki, sl * P:(sl + 1) * P],
                                        qT[:, ki, lo:lo + NCH],
                                        start=(ki == 0),
                                        stop=(ki == CK - 1))
                                nc.scalar.activation(
                                    att[:, s_idx, lo:lo + NCH], a_ps[:],
                                    AF.Exp, scale=ATT_SCALE)
                                nc.tensor.matmul(
                                    sums_ps[sub][:], ones_b[:],
                                    att[:, s_idx, lo:lo + NCH],
                                    start=(order == 0),
                                    stop=(order == S - 1))
                # softmax denominators -> broadcast reciprocal
                for sub in range(2):
                    lo = sub * NCH
                    srow = pb.tile([1, NCH], F32, tag="srow", bufs=2)
                    nc.scalar.activation(srow[:], sums_ps[sub][:], AF.Copy)
                    nc.vector.reciprocal(srow[:], srow[:])
                    srow_r = pb.tile([1, NCH], F32R, tag="srowr", bufs=2)
                    nc.vector.tensor_copy(srow_r[:], srow[:])
                    rb_ps = pst()
                    nc.tensor.matmul(rb_ps[:], ones_row[:], srow_r[:],
                                     start=True, stop=True)
                    nc.vector.tensor_copy(recip_b[:, lo:lo + NCH], rb_ps[:])

                v_sb = pb.tile([P, S, C], BF16, tag="vsb")
                for th in range(2):
                    for g in range(2):
                        for sl in range(4):
                            nc.sync.dma_start(
                                v_sb[:, g * 8 + th * 4 + sl, :],
                                kv_v_all[th][g, sl])
                wp_sb = pb.tile([P, CK, C], BF16, tag="wpb")
                nc.sync.dma_start(wp_sb[:], wp[:])

                yT = pb.tile([P, CK, TQ], BF16, tag="yT")
                for sub in range(2):
                    lo = sub * NCH
                    y_ps = [pst() for _ in range(CK)]
                    for s in range(S):
                        for cti in range(CK):
                            nc.tensor.matmul(
                                y_ps[cti][:],
                                v_sb[:, s, cti * P:(cti + 1) * P],
                                att[:, s, lo:lo + NCH],
                                start=(s == 0), stop=(s == S - 1))
                    for cti in range(CK):
                        nc.vector.tensor_mul(
                            yT[:, cti, lo:lo + NCH], y_ps[cti][:],
                            recip_b[:, lo:lo + NCH])
                # proj + residual (bf16 residual stream); LN2 stats matmuls
                # interleave right behind each chunk's evictions
                x2b = gp.tile([P, CK, TQ], BF16, tag="x2b")
                stats2 = []
                for sub in range(2):
                    lo = sub * NCH
                    for jo in range(CK):
                        z_ps = pst()
                        for ki in range(CK):
                            nc.tensor.matmul(
                                z_ps[:], wp_sb[:, jo, ki * P:(ki + 1) * P],
                                yT[:, ki, lo:lo + NCH],
                                start=(ki == 0), stop=(ki == CK - 1))
                        nc.vector.scalar_tensor_tensor(
                            x2b[:, jo, lo:lo + NCH], z_ps[:],
                            bp_t[:, jo:jo + 1], xt[:, jo, lo:lo + NCH],
                            op0=ALU.add, op1=ALU.add)
                    stats2.append(stats_mms(pb, x2b, sub, sq_on_dve=False))
            pAB_cm.__exit__(None, None, None)

            # ===== phase D: LN2 (folded) + MLP + final residual =====
            with tc.tile_pool(name="pd", bufs=1) as pd:
                r2_b = pd.tile([P, TQ], F32, tag="rb")
                mur2_b = pd.tile([P, TQ], F32, tag="murb")
                rows2 = ln_rows(pd, stats2)
                ln_bcast(rows2, r2_b, mur2_b)
                out_acc = pd.tile([P, CK, TQ], F32, tag="oacc")
                for hhalf in range(2):
                    gel = pd.tile([P, HK // 2, TQ], BF16, tag="gel", bufs=2)
                    for jl in range(HK // 2):
                        jh = hhalf * (HK // 2) + jl
                        w2_sb = pd.tile([P, C], BF16, tag="w2", bufs=3)
                        nc.sync.dma_start(w2_sb[:], w2[jh])
                        for sub in range(2):
                            lo = sub * NCH
                            m_ps = pst()
                            for ki in range(CK):
                                nc.tensor.matmul(
                                    m_ps[:], w2_sb[:, ki * P:(ki + 1) * P],
                                    x2b[:, ki, lo:lo + NCH],
                                    start=(ki == 0), stop=(ki == CK - 1))
                            tmp = pd.tile([P, NCH], F32, tag="ktmp", bufs=3)
                            nc.vector.tensor_mul(tmp[:], m_ps[:],
                                                 r2_b[:, lo:lo + NCH])
                            t2 = pd.tile([P, NCH], F32, tag="t2", bufs=3)
                            nc.vector.scalar_tensor_tensor(
                                t2[:], mur2_b[:, lo:lo + NCH],
                                s2n_t[:, jh:jh + 1], tmp[:],
                                op0=ALU.mult, op1=ALU.add)
                            nc.scalar.activation(
                                gel[:, jl, lo:lo + NCH], t2[:],
                                AF.Gelu_apprx_tanh, bias=c2_t[:, jh:jh + 1])
                    for jo in range(CK):
                        wm_sb = pd.tile([P, H // 2], BF16, tag="wm", bufs=2)
                        nc.sync.dma_start(
                            wm_sb[:],
                            wm[jo, :,
                               hhalf * (H // 2):(hhalf + 1) * (H // 2)])
                        for sub in range(2):
                            lo = sub * NCH
                            o_ps = pst()
                            for kk in range(HK // 2):
                                nc.tensor.matmul(
                                    o_ps[:], wm_sb[:, kk * P:(kk + 1) * P],
                                    gel[:, kk, lo:lo + NCH],
                                    start=(kk == 0), stop=(kk == HK // 2 - 1))
                            if hhalf == 0:
                                nc.vector.scalar_tensor_tensor(
                                    out_acc[:, jo, lo:lo + NCH], o_ps[:],
                                    bm_t[:, jo:jo + 1],
                                    x2b[:, jo, lo:lo + NCH],
                                    op0=ALU.add, op1=ALU.add)
                            else:
                                o_sb = pd.tile([P, NCH], F32, tag="oev",
                                               bufs=4)
                                nc.vector.tensor_add(
                                    o_sb[:], o_ps[:],
                                    out_acc[:, jo, lo:lo + NCH])
                                nc.sync.dma_start(
                                    out_t[jo * P:(jo + 1) * P, lo:lo + NCH],
                                    o_sb[:])
    nc.finalize()
    return nc


_prog = None
_prog_bias = None


def _get_prog(qkv_bias):
    global _prog, _prog_bias
    if _prog is None or _prog_bias != qkv_bias:
        _prog = _build(qkv_bias=qkv_bias)
        _prog_bias = qkv_bias
    return _prog


def _pack_weights(ln1_g, ln1_b, w_attn, b_attn, w_proj, b_proj,
                  ln2_g, ln2_b, w_fc, b_fc, w_mlp_proj, b_mlp_proj):
    f = np.float32
    bf = ml_dtypes.bfloat16
    W1 = (ln1_g[:, None] * w_attn).astype(f)            # [C, 3C]
    c1 = (ln1_b @ w_attn + b_attn).astype(f)            # [3C]
    s1 = W1.sum(axis=0).astype(f)                       # column sums [3C]
    kq = np.concatenate([W1[:, C:2 * C], W1[:, :C]], axis=1)  # k then q
    w1kq_h = np.ascontiguousarray(
        kq.reshape(CK, P, 2 * CK, P).transpose(1, 2, 0, 3)
        .reshape(P, 2 * CK, C)).astype(bf)
    w1v_h = np.ascontiguousarray(
        W1[:, 2 * C:].reshape(CK, P, C).transpose(1, 0, 2)).astype(bf)
    wp_h = np.ascontiguousarray(
        w_proj.astype(f).reshape(CK, P, CK, P).transpose(1, 2, 0, 3)
        .reshape(P, CK, C)).astype(bf)
    W2 = (ln2_g[:, None] * w_fc).astype(f)              # [C, H]
    c2v = (ln2_b @ w_fc + b_fc).astype(f)               # [H]
    s2 = W2.sum(axis=0).astype(f)                       # [H]
    w2_h = np.ascontiguousarray(
        W2.reshape(CK, P, HK, P).transpose(2, 1, 0, 3).reshape(HK, P, C)
    ).astype(bf)
    wm_h = np.ascontiguousarray(
        w_mlp_proj.astype(f).reshape(HK, P, CK, P).transpose(2, 1, 0, 3)
        .reshape(CK, P, H)).astype(bf)

    def colT(v):
        return np.ascontiguousarray(v.reshape(-1, P).T).astype(f)

    return {
        "w1kq": w1kq_h,
        "w1v": w1v_h,
        "wp": wp_h,
        "w2": w2_h,
        "wm": wm_h,
        "s1kn": colT(-s1[C:2 * C]),
        "s1qn": colT(-s1[:C]),
        "s1vnb": np.ascontiguousarray(
            np.broadcast_to(-s1[2 * C:], (P, C))).astype(f),
        "s2n": colT(-s2),
        "c1k": colT(c1[C:2 * C]),
        "c1q": colT(c1[:C]),
        "c1vb": np.ascontiguousarray(
            np.broadcast_to(c1[2 * C:], (P, C))).astype(f),
        "bp": colT(b_proj.astype(f)),
        "c2": colT(c2v),
        "bm": colT(b_mlp_proj.astype(f)),
        "onc_b": np.ones((P, 1), ml_dtypes.bfloat16),
        "onr_r": np.ones((1, P), f),
        "warm": np.zeros((1, 64), ml_dtypes.bfloat16),
    }, bool(np.any(c1[:2 * C] != 0.0))


def kernel(x, ln1_g, ln1_b, w_attn, b_attn, w_proj, b_proj,
           ln2_g, ln2_b, w_fc, b_fc, w_mlp_proj, b_mlp_proj,
           _trace=False):
    x = np.asarray(x, np.float32)
    shared, qkv_bias = _pack_weights(
        np.asarray(ln1_g, np.float32), np.asarray(ln1_b, np.float32),
        np.asarray(w_attn, np.float32), np.asarray(b_attn, np.float32),
        np.asarray(w_proj, np.float32), np.asarray(b_proj, np.float32),
        np.asarray(ln2_g, np.float32), np.asarray(ln2_b, np.float32),
        np.asarray(w_fc, np.float32), np.asarray(b_fc, np.float32),
        np.asarray(w_mlp_proj, np.float32),
        np.asarray(b_mlp_proj, np.float32))

    in_maps = []
    for core in range(N_CORES):
        b, h = core // 2, core % 2
        xTc = (x[b, h * TQ:(h + 1) * TQ].T.reshape(CK, P, TQ)
               .transpose(1, 0, 2))                      # [P, CK, TQ]
        xsp = np.ascontiguousarray(
            xTc.reshape(P, CK, 2, NCH).transpose(2, 0, 1, 3))
        in_maps.append({
            "xbf": xsp.astype(ml_dtypes.bfloat16),
            **shared,
        })

    nc = _get_prog(qkv_bias)
    res = run_bass_kernel_spmd(nc, in_maps, list(range(N_CORES)),
                               trace=_trace)
    out = np.empty_like(x)
    for core in range(N_CORES):
        b, h = core // 2, core % 2
        out[b, h * TQ:(h + 1) * TQ] = res.results[core]["out_t"].T
    if _trace:
        kernel._last_exec_time_ns = res.exec_time_ns
        kernel._last_profile = res.profile_json
        if res.instructions_and_trace is not None:
            kernel._last_trace_path = res.instructions_and_trace[1]
    return out
